# revision 46
# baseline (speedup 1.0000x reference)
"""Trainium2 Bass kernel for nn_HGraphConv (4-hop masked-softmax graph conv).

Math per hop k:  out_k = softmax(where(m_k, E_k, NEG), axis=1) @ (x @ W_k)
Final:           concat(out_0..out_3, axis=2) + bias

Device strategy (data-parallel over batch B=64 across 8 cores, 8 batches/core):
  - Host packs E'_k = where(m_k, E_k, -100).T in fp16 (k=1..3). exp(-100)
    underflows to exactly 0 in fp16, so the mask multiply disappears and the
    masked softmax is exact (no empty mask rows for this graph; host fallback
    patches them otherwise).
  - All matmuls run in fp16 (1 PE cycle/row vs 4 for fp32) with fp32 PSUM
    accumulation:
      * One 512-wide matmul per (j-chunk, batch) computes H for all four hops
        at once: stationary x^T chunk, moving [W0|W1|W2|W3]. The hop-0 slice
        is directly out_0 (A_0 = I since m_0 = I).
      * out_k[i,(b,f)] += P_k^T-tile @ H-tile accumulated over j in PSUM; the
        row sum Z[i] comes from one extra N=1 matmul vs a ones-vector reusing
        the already-loaded stationary tile.
  - Engine split: PE matmuls / ACT exp (fp16 in-place) / DVE evictions
    (psum * (1/Z) + bias fused, fp16 out) / Pool H-copies.
  - Outputs staged fp16 (halves write traffic); host casts to fp32.
"""

import os
import sys

import numpy as np

sys.path.insert(0, "/opt/trn_rl_repo")
sys.path.insert(0, "/opt/trn_rl_repo/concourse")

import concourse.bass as bass  # noqa: E402
import concourse.mybir as mybir  # noqa: E402
import concourse.tile as tile  # noqa: E402
import concourse.bass_utils as _bu  # noqa: E402
import concourse.bass2jax as _b2j  # noqa: E402
from concourse.bass_utils import run_bass_kernel_spmd  # noqa: E402

# ---------------------------------------------------------------------------
# Workaround for this walrus build: the TRN2 ISA has exactly one sync-wait
# slot per 64B instruction, and this compiler errors ("Too many sync wait
# commands") instead of splitting multi-wait instructions emitted by Tile.
# Split them ourselves at the BIR-JSON level: hoist all but one wait onto
# single-wait NoOps inserted right before the instruction on the same engine
# queue (queue waits execute in order, so this is semantically identical).
# ---------------------------------------------------------------------------
import json as _json  # noqa: E402


def _split_multi_waits_json(bir_json):
    if isinstance(bir_json, (bytes, bytearray)):
        m = _json.loads(bir_json.decode())
    else:
        m = _json.loads(bir_json)
    ctr = 0
    for fn in m["functions"]:
        for blk in fn["blocks"]:
            out = []
            for inst in blk["instructions"]:
                si = inst.get("sync_info")
                if si:
                    ws = si.get("on_wait") or []
                    if len(ws) > 1:
                        for w in ws[:-1]:
                            ctr += 1
                            out.append(
                                {
                                    "debug": inst.get("debug", 0),
                                    "engine": inst["engine"],
                                    "ins": [],
                                    "name": f"WX-{ctr}",
                                    "opcode": "NoOp",
                                    "outs": [],
                                    "text_hint": "split_wait",
                                    "sync_info": {
                                        "on_update": [],
                                        "on_wait": [w],
                                    },
                                }
                            )
                        si["on_wait"] = [ws[-1]]
                    us = si.get("on_update") or []
                    if len(us) > 1:
                        raise RuntimeError(
                            f"multi-update inst {inst['name']}: unsupported"
                        )
                out.append(inst)
            blk["instructions"] = out
    return _json.dumps(m).encode()


_orig_compile_bir_kernel = _bu.compile_bir_kernel.__wrapped__ if hasattr(
    _bu.compile_bir_kernel, "__wrapped__"
) else _bu.compile_bir_kernel


def _patched_compile_bir_kernel(bir_json, tmpdir, neff_name="file.neff"):
    return _orig_compile_bir_kernel(
        _split_multi_waits_json(bir_json), tmpdir, neff_name
    )


_bu.compile_bir_kernel = _patched_compile_bir_kernel
if hasattr(_b2j, "compile_bir_kernel"):
    _b2j.compile_bir_kernel = _patched_compile_bir_kernel

N_CORES = 8
B = 64
N = 1024
F = 128
HOPS = 4
NEG = -9.0e15

# filled by kernel() for test.py to read
last_run_info = {}


def build_nc(b_local: int, n: int, f: int = 128, reps: int = 1, variant: str = ""):
    """Build the per-core Bass module.

    b_local: batches per core (8).  n: graph nodes.  f: feature dim (=128).
    """
    P = 128
    assert f == 128 and n % P == 0
    nch = n // P                # number of 128-row chunks (j and i)
    khops = HOPS - 1            # hops that need attention (1..3)
    bg = 4                      # batches per 512-wide eviction half
    n_half = b_local // bg      # halves of the (b,f) column space
    assert b_local == bg * n_half

    nc = bass.Bass()
    fp32 = mybir.dt.float32
    fp16 = mybir.dt.float16
    fp8 = mybir.dt.float8e4
    DR = mybir.MatmulPerfMode.DoubleRow
    xt_d = nc.dram_tensor("xt", [b_local, f, n], fp16, kind="ExternalInput")
    ep_d = nc.dram_tensor("ep", [khops, n, n], fp16, kind="ExternalInput")
    wc_d = nc.dram_tensor("wc", [f, HOPS * f], fp16, kind="ExternalInput")
    bias_d = nc.dram_tensor("bias", [HOPS * f], fp32, kind="ExternalInput")
    b16_d = nc.dram_tensor("bias16", [HOPS * f], fp16, kind="ExternalInput")
    out_d = nc.dram_tensor(
        "out", [nch, P, HOPS, b_local, f], fp16, kind="ExternalOutput"
    )

    with tile.TileContext(nc) as tc:
        with (
            tc.tile_pool(name="const", bufs=1) as const,
            tc.tile_pool(name="et", bufs=2) as etp,
            tc.tile_pool(name="et8", bufs=2) as et8p,
            tc.tile_pool(name="hh", bufs=1) as hhp,
            tc.tile_pool(name="stage", bufs=4) as stp,
            tc.tile_pool(name="zi", bufs=2) as zip_,
            tc.tile_pool(name="psA", bufs=3, space="PSUM") as psA,
        ):
            # ---- constants (loaded once, outside the reps loop) ----
            xt = const.tile([P, b_local, n], fp16)
            nc.sync.dma_start(out=xt, in_=xt_d[:].rearrange("b f j -> f b j"))
            wc = const.tile([P, HOPS * f], fp16)
            nc.sync.dma_start(out=wc, in_=wc_d[:])
            ones = const.tile([P, 8], fp16)
            nc.vector.memset(ones, 1.0)
            ones8 = const.tile([P, 2, 16], fp8)
            nc.vector.memset(ones8, 1.0)
            # one-hot rows for the PE-side Z transpose ([1,128] chunks -> zt cols)
            eye8 = const.tile([1, nch, nch], fp16)
            nc.vector.memset(eye8, 0.0)
            for c in range(nch):
                nc.vector.memset(eye8[:, c, c:c + 1], 1.0)
            # [bias0 | bias1] for the fused hop-0/1 h_build eviction (2 batches):
            # folding bias into H makes hop-1's eviction a pure 1/Z scale
            br01 = const.tile([P, 2, 2, f], fp32)
            for b2 in range(2):
                for kb in range(2):
                    bsl = bias_d[kb * f:(kb + 1) * f]
                    nc.sync.dma_start(
                        out=br01[:, b2, kb, :],
                        in_=bass.AP(
                            tensor=bsl.tensor, offset=bsl.offset,
                            ap=[[0, P], [1, f]],
                        ),
                    )
            # full bias replicated across partitions/batches (sttbias variant)
            br = const.tile([P, HOPS, b_local, f], fp32)
            for kb in range(HOPS):
                bsl = bias_d[kb * f:(kb + 1) * f]
                nc.sync.dma_start(
                    out=br[:, kb],
                    in_=bass.AP(
                        tensor=bsl.tensor, offset=bsl.offset,
                        ap=[[0, P], [0, b_local], [1, f]],
                    ),
                )
            # fp16 bias rows (hops 2,3) for the Z (x) bias outer-product
            bb16 = const.tile([1, 2, bg, f], fp16)
            for kb in range(2):
                bsl = b16_d[(kb + 2) * f:(kb + 3) * f]
                nc.sync.dma_start(
                    out=bb16[:, kb],
                    in_=bass.AP(
                        tensor=bsl.tensor, offset=bsl.offset,
                        ap=[[0, 1], [0, bg], [1, f]],
                    ),
                )

            for _rep in range(reps):
                def load_exp(kk):
                    """DMA E'_k^T (fp16, pre-masked) and exp it.

                    Hops 1,2 (kk=0,1): exp in place, fp16 (matmul dtype).
                    Hop 3 (kk=2): exp into an fp8e4 tile for DoubleRow.
                    """
                    et = etp.tile([P, nch, n], fp16, tag="et")
                    e8 = None if kk < 2 else et8p.tile(
                        [P, nch, n], fp8, tag="et8"
                    )
                    hc = nch // 2
                    for half in range(2):
                        sl = slice(half * hc, (half + 1) * hc)
                        nc.sync.dma_start(
                            out=et[:, sl],
                            in_=ep_d[kk, half * hc * P:(half + 1) * hc * P]
                            .rearrange("(c p) i -> p c i", p=P),
                        )
                        nc.scalar.activation(
                            out=(et if kk < 2 else e8)[:, sl],
                            in_=et[:, sl],
                            func=mybir.ActivationFunctionType.Exp,
                        )
                    return et if kk < 2 else e8

                if _rep == 0:
                    _pending = load_exp(0)
                ets = [None] * khops
                ets[0] = _pending

                def z_block(et, kk, on_act):
                    """Row-sums Z of P_k: ones-stationary chains -> [1, 512]
                    PSUM x2 -> fp16 copy (DVE or ACT) -> PE one-hot transpose
                    -> reciprocal. Returns (zs16, zinv8)."""
                    if "noz" in variant:
                        zs16 = zip_.tile([1, n], fp16, tag="zs")
                        nc.vector.memset(zs16, 1.0)
                        zinv8 = zip_.tile([P, nch], fp32, tag="zinv")
                        nc.vector.memset(zinv8, 1.0)
                        return zs16, zinv8
                    f8 = kk == 2
                    zps = [
                        psA.tile([1, n // 2], fp32, tag="z", bufs=2,
                                 name=f"zps_{kk}_{zz}")
                        for zz in range(2)
                    ]
                    if f8:
                        for jp in range(nch // 2):
                            st_, sp_ = (jp == 0), (jp == nch // 2 - 1)
                            for zz in range(2):
                                nc.tensor.matmul(
                                    zps[zz],
                                    ones8[:, :, 0:1],
                                    et[:, 2 * jp:2 * jp + 2,
                                       zz * (n // 2):(zz + 1) * (n // 2)],
                                    start=st_, stop=sp_, perf_mode=DR,
                                )
                    else:
                        for jc in range(nch):
                            st_, sp_ = (jc == 0), (jc == nch - 1)
                            for zz in range(2):
                                nc.tensor.matmul(
                                    zps[zz],
                                    ones[:, 0:1],
                                    et[:, jc, zz * (n // 2):(zz + 1) * (n // 2)],
                                    start=st_, stop=sp_,
                                )
                    zs16 = zip_.tile([1, n], fp16, tag="zs")
                    for zz in range(2):
                        sl = zs16[:, zz * (n // 2):(zz + 1) * (n // 2)]
                        if on_act:
                            nc.scalar.copy(out=sl, in_=zps[zz])
                        else:
                            nc.vector.tensor_scalar_add(sl, zps[zz], 0.0)
                    zinv8 = zip_.tile([P, nch], fp32, tag="zinv")
                    if "nozt" in variant:
                        nc.vector.memset(zinv8, 1.0)
                    else:
                        # transpose Z to per-partition layout on the PE: 8
                        # one-hot outer products accumulated into one PSUM
                        # region
                        zt = psA.tile([P, nch], fp32, tag="z", bufs=2,
                                      name=f"zt_{kk}")
                        for c in range(nch):
                            nc.tensor.matmul(
                                zt,
                                zs16[:, c * P:(c + 1) * P],
                                eye8[:, c],
                                start=(c == 0),
                                stop=(c == nch - 1),
                            )
                        nc.vector.reciprocal(out=zinv8, in_=zt)
                    return zs16, zinv8

                # hop-1 Z computed before h_build: PE only needs exp(E1),
                # and its fp16 staging lands early in the DVE queue
                zinfo = [None] * khops
                zinfo[0] = z_block(ets[0], 0, on_act=False)

                # ---- H for all hops in one 512-wide matmul per (jc, b);
                #      hop-0 slice (plus bias) is directly out_0, staged in
                #      hh; hop-1 H gets bias folded (pure-scale eviction);
                #      hop-2 H raw fp16; hop-3 H in fp8 for DoubleRow ----
                hh = hhp.tile([P, nch, b_local, 3, f], fp16, tag="hh")
                hh8 = hhp.tile([P, nch, b_local, f], fp8, tag="hh8")
                for jc in range(nch):
                    for bb in range(0, b_local, 2):
                        ps = psA.tile([P, 2 * HOPS * f], fp32, tag="A")
                        for db in range(2):
                            nc.tensor.matmul(
                                ps[:, db * HOPS * f:(db + 1) * HOPS * f],
                                xt[:, bb + db, jc * P:(jc + 1) * P],
                                wc,
                                start=True,
                                stop=True,
                            )
                        psv = ps.rearrange("p (b k f) -> p b k f", b=2, k=HOPS)
                        nc.vector.tensor_tensor(
                            out=hh[:, jc, bb:bb + 2, 0:2, :],
                            in0=psv[:, :, 0:2, :],
                            in1=br01,
                            op=mybir.AluOpType.add,
                        )
                        nc.scalar.copy(
                            out=hh[:, jc, bb:bb + 2, 2, :],
                            in_=psv[:, :, 2, :],
                        )
                        nc.scalar.copy(
                            out=hh8[:, jc, bb:bb + 2, :],
                            in_=psv[:, :, 3, :],
                        )
                    nc.sync.dma_start(
                        out=out_d[jc, :, 0], in_=hh[:, jc, :, 0, :]
                    )

                # ---- hops 1..3 ----
                for kk in range(khops):
                    if kk > 0:
                        # Z for this hop first (ACT-side staging for the fp8
                        # hop must precede the next exp in the ACT queue)
                        zinfo[kk] = z_block(ets[kk], kk, on_act=(kk == 2))
                    if kk + 1 < khops:
                        ets[kk + 1] = load_exp(kk + 1)
                    elif _rep + 1 < reps:
                        # prefetch next rep's hop-1 E during this rep's hop 3
                        _pending = load_exp(0)
                    et = ets[kk]
                    k = kk + 1
                    f8 = kk == 2
                    zs16, zinv8 = zinfo[kk]

                    has_outer = (kk > 0 and "noouter" not in variant
                                 and "sttbias" not in variant)
                    for ib in range(nch):
                        pos = psA.tile([P, b_local * f], fp32, tag="A",
                                       name=f"pos_{ib}")
                        if f8:
                            for jp in range(nch // 2):
                                lhsT = et[:, 2 * jp:2 * jp + 2,
                                          ib * P:(ib + 1) * P]
                                st_ = (jp == 0)
                                sp_ = (jp == nch // 2 - 1) and not has_outer
                                for h in range(n_half):
                                    nc.tensor.matmul(
                                        pos[:, h * bg * f:(h + 1) * bg * f],
                                        lhsT,
                                        hh8[:, 2 * jp:2 * jp + 2,
                                            h * bg:(h + 1) * bg, :],
                                        start=st_,
                                        stop=sp_,
                                        perf_mode=DR,
                                    )
                        else:
                            slot = 1 if kk == 0 else 2
                            for jc in range(nch):
                                lhsT = et[:, jc, ib * P:(ib + 1) * P]
                                st_ = (jc == 0)
                                sp_ = (jc == nch - 1) and not has_outer
                                for h in range(n_half):
                                    nc.tensor.matmul(
                                        pos[:, h * bg * f:(h + 1) * bg * f],
                                        lhsT,
                                        hh[:, jc, h * bg:(h + 1) * bg, slot, :],
                                        start=st_,
                                        stop=sp_,
                                    )
                        if has_outer:
                            # bias via Z (x) bias_k outer-product (K=1 matmul):
                            # (P@H + Z*b) / Z == out + b
                            for h in range(n_half):
                                nc.tensor.matmul(
                                    pos[:, h * bg * f:(h + 1) * bg * f],
                                    zs16[:, ib * P:(ib + 1) * P],
                                    bb16[:, kk - 1].rearrange("q a b -> q (a b)"),
                                    start=False,
                                    stop=True,
                                )
                        # eviction: pure 1/Z scale when bias is already in
                        # PSUM/H, else fused scale+bias stt (sttbias variant)
                        st = stp.tile([P, b_local * f], fp16, tag="stage")
                        if "sttbias" in variant and kk > 0:
                            nc.vector.scalar_tensor_tensor(
                                out=st,
                                in0=pos,
                                scalar=zinv8[:, ib:ib + 1],
                                in1=br[:, k].rearrange("p a b -> p (a b)"),
                                op0=mybir.AluOpType.mult,
                                op1=mybir.AluOpType.add,
                            )
                        elif ((kk == 2 and ib % 2 == 1)
                              or ("sttbias" in variant and ib % 2 == 1)):
                            nc.scalar.activation(
                                out=st,
                                in_=pos,
                                func=mybir.ActivationFunctionType.Copy,
                                scale=zinv8[:, ib:ib + 1],
                            )
                        else:
                            nc.vector.tensor_scalar_mul(
                                st, pos, zinv8[:, ib:ib + 1]
                            )
                        nc.sync.dma_start(
                            out=out_d[ib, :, k],
                            in_=st.rearrange("p (b f) -> p b f", b=b_local),
                        )
    return nc


_nc_cache = {}


def _get_nc(b_local, n, f):
    key = (b_local, n, f)
    if key not in _nc_cache:
        _nc_cache[key] = build_nc(b_local, n, f)
    return _nc_cache[key]


def _run(x, W, Es, bias, ms, n_cores, trace=False):
    """x:[B,N,F] W:[4,F,F] Es:[E1,E2,E3] ms:[m1,m2,m3] (hop-0 handled as identity)."""
    b, n, f = x.shape
    b_local = b // n_cores
    nc = _get_nc(b_local, n, f)

    # fold the mask into E on the host: exp(-100) == 0 exactly in fp16
    ep = np.ascontiguousarray(
        np.stack(
            [np.where(m, e, -100.0).T for e, m in zip(Es, ms)]
        ).astype(np.float16)
    )
    wc = np.ascontiguousarray(
        np.concatenate([W[k] for k in range(HOPS)], axis=1).astype(np.float16)
    )
    bias = np.ascontiguousarray(bias.astype(np.float32))

    in_maps = []
    for c in range(n_cores):
        xs = x[c * b_local:(c + 1) * b_local]          # [b_local, n, f]
        xts = np.ascontiguousarray(xs.transpose(0, 2, 1).astype(np.float16))
        in_maps.append({"xt": xts, "ep": ep, "wc": wc, "bias": bias,
                        "bias16": bias.astype(np.float16)})

    last_run_info["nc"] = nc
    last_run_info["in_maps"] = in_maps
    res = run_bass_kernel_spmd(
        nc, in_maps, core_ids=list(range(n_cores)), trace=trace
    )
    last_run_info["exec_time_ns"] = res.exec_time_ns
    last_run_info["trace"] = res.instructions_and_trace

    out = np.empty((b, n, HOPS * f), dtype=np.float32)
    for c in range(n_cores):
        od = res.results[c]["out"]          # [nch, P, HOPS, b_local, f]
        oc = od.transpose(3, 0, 1, 2, 4).reshape(b_local, n, HOPS * f)
        out[c * b_local:(c + 1) * b_local] = oc.astype(np.float32)
    return out


def build_null_nc(b_local: int, n: int, f: int = 128):
    """Same external tensors as build_nc but ~no device work — used to
    subtract host/transfer/dispatch overhead when estimating HW exec time."""
    P = 128
    khops = HOPS - 1
    nc = bass.Bass()
    fp32 = mybir.dt.float32
    fp16 = mybir.dt.float16
    nc.dram_tensor("xt", [b_local, f, n], fp16, kind="ExternalInput")
    nc.dram_tensor("ep", [khops, n, n], fp16, kind="ExternalInput")
    wc_d = nc.dram_tensor("wc", [f, HOPS * f], fp16, kind="ExternalInput")
    nc.dram_tensor("bias", [HOPS * f], fp32, kind="ExternalInput")
    nc.dram_tensor("bias16", [HOPS * f], fp16, kind="ExternalInput")
    nch = n // P
    out_d = nc.dram_tensor(
        "out", [nch, P, HOPS, b_local, f], fp16, kind="ExternalOutput"
    )
    with tile.TileContext(nc) as tc:
        with tc.tile_pool(name="p", bufs=1) as pool:
            t = pool.tile([P, 8], fp16)
            nc.sync.dma_start(out=t, in_=wc_d[:, 0:8])
            nc.sync.dma_start(out=out_d[0, :, 0, 0, 0:8], in_=t)
    return nc


def time_exec(iters=3):
    """Re-execute the last-run kernel and a null kernel; return
    (min_real_s, min_null_s). Uses identical input tensors so transfer and
    dispatch overhead cancels in the difference."""
    import time as _t

    nc = last_run_info["nc"]
    in_maps = last_run_info["in_maps"]
    n_cores = len(in_maps)
    reals, nulls = [], []
    for _ in range(iters):
        t0 = _t.time()
        run_bass_kernel_spmd(nc, in_maps, core_ids=list(range(n_cores)))
        reals.append(_t.time() - t0)
    b_local, f, n = in_maps[0]["xt"].shape
    nnc = build_null_nc(b_local, n, f)
    for _ in range(iters):
        t0 = _t.time()
        run_bass_kernel_spmd(nnc, in_maps, core_ids=list(range(n_cores)))
        nulls.append(_t.time() - t0)
    return min(reals), min(nulls), reals, nulls


def bench_exec(nc, in_maps, iters=10):
    """Device-resident repeated execution of the compiled kernel; returns
    per-call wall times (s) with inputs pre-staged on the 8 cores so only
    dispatch + device execution is measured."""
    import time as _t

    import jax
    import jax.numpy as jnp
    import mybir  # noqa: F401  # (ensure concourse paths set)
    from jax.experimental.shard_map import shard_map
    from jax.sharding import Mesh, PartitionSpec

    import concourse.mybir as mb
    from concourse import bass2jax as B

    B.install_neuronx_cc_hook()
    n_cores = len(in_maps)
    partition_name = (
        nc.partition_id_tensor.name if nc.partition_id_tensor else None
    )
    in_names, out_names, out_avals, zero_shapes = [], [], [], []
    for alloc in nc.m.functions[0].allocations:
        if not isinstance(alloc, mb.MemoryLocationSet):
            continue
        name = alloc.memorylocations[0].name
        if alloc.kind == "ExternalInput":
            if name != partition_name:
                in_names.append(name)
        elif alloc.kind == "ExternalOutput":
            shape = tuple(alloc.tensor_shape)
            dtype = mb.dt.np(alloc.dtype)
            out_names.append(name)
            out_avals.append(jax.core.ShapedArray(shape, dtype))
            zero_shapes.append((shape, dtype))
    n_params = len(in_names)
    all_in_names = list(in_names) + list(out_names)
    if partition_name is not None:
        all_in_names.append(partition_name)
    donate = tuple(range(n_params, n_params + len(out_names)))

    def _body(*args):
        operands = list(args)
        if partition_name is not None:
            operands.append(B.partition_id_tensor())
        outs = B._bass_exec_p.bind(
            *operands,
            out_avals=tuple(out_avals),
            in_names=tuple(all_in_names),
            out_names=tuple(out_names),
            lowering_input_output_aliases=(),
            sim_require_finite=True,
            sim_require_nnan=True,
            nc=nc,
        )
        return tuple(outs)

    devices = jax.devices()[:n_cores]
    mesh = Mesh(np.asarray(devices), ("core",))
    in_specs = (PartitionSpec("core"),) * (n_params + len(out_names))
    out_specs = (PartitionSpec("core"),) * len(out_names)
    fn = jax.jit(
        shard_map(
            _body, mesh=mesh, in_specs=in_specs, out_specs=out_specs,
            check_rep=False,
        ),
        donate_argnums=donate,
        keep_unused=True,
    )
    sh = jax.sharding.NamedSharding(mesh, PartitionSpec("core"))
    dev_in = [
        jax.device_put(
            np.concatenate([np.asarray(m[nm]) for m in in_maps], axis=0), sh
        )
        for nm in in_names
    ]

    def zeros():
        return [
            jax.device_put(
                jnp.zeros((n_cores * s[0],) + tuple(s[1:]), dt), sh
            )
            for (s, dt) in zero_shapes
        ]

    # warm up (compile + first exec)
    outs = fn(*dev_in, *zeros())
    jax.block_until_ready(outs)
    times = []
    for _ in range(iters):
        z = zeros()
        jax.block_until_ready(z)
        t0 = _t.perf_counter()
        outs = fn(*dev_in, *z)
        jax.block_until_ready(outs)
        times.append(_t.perf_counter() - t0)
    return times


def kernel(**inputs) -> np.ndarray:
    x = np.asarray(inputs["x"], dtype=np.float32)
    W = np.asarray(inputs["W"], dtype=np.float32)
    Es = [np.asarray(inputs[f"E{i}"], dtype=np.float32) for i in range(4)]
    bias = np.asarray(inputs["bias"], dtype=np.float32)
    ms = [np.asarray(inputs[f"m{i}"]).astype(bool) for i in range(4)]

    trace = bool(int(os.environ.get("HGRAPH_TRACE", "0")))
    out = _run(x, W, Es[1:], bias, ms[1:], N_CORES, trace=trace)

    f = W.shape[2]
    n = x.shape[1]
    # Safety net 1: hop 0 assumes m0 == I (structurally true for this module).
    if not np.array_equal(ms[0], np.eye(n, dtype=bool)):
        s0 = np.where(ms[0], Es[0], NEG)
        s0 = s0 - s0.max(axis=1, keepdims=True)
        p0 = np.exp(s0)
        a0 = p0 / p0.sum(axis=1, keepdims=True)
        h0 = np.einsum("bnf,fo->bno", x, W[0])
        out[:, :, 0:f] = np.einsum("ij,bjo->bio", a0, h0) + bias[None, None, :f]
    # Safety net 2: all-masked rows (softmax -> uniform; device would give NaN).
    for k in range(1, 4):
        empty = ~ms[k].any(axis=1)
        if empty.any():
            hk = np.einsum("bnf,fo->bno", x, W[k])
            unif = hk.mean(axis=1)  # [B, f]
            idx = np.where(empty)[0]
            out[:, idx, k * f:(k + 1) * f] = unif[:, None, :] + bias[None, None, k * f:(k + 1) * f]
    return out


# revision 47
# speedup vs baseline: 20.5370x; 20.5370x over previous
"""Trainium2 Bass kernel for nn_HGraphConv (4-hop masked-softmax graph conv).

Math per hop k:  out_k = softmax(where(m_k, E_k, NEG), axis=1) @ (x @ W_k)
Final:           concat(out_0..out_3, axis=2) + bias

Device strategy (data-parallel over batch B=64 across 8 cores, 8 batches/core):
  - Host packs E'_k = where(m_k, E_k, -100).T in fp16 (k=1..3). exp(-100)
    underflows to exactly 0, so the mask multiply disappears and the masked
    softmax is exact (no empty mask rows for this graph; host fallback
    patches them otherwise).
  - Matmuls run in fp16 (1 PE cycle/row vs 4 for fp32) with fp32 PSUM
    accumulation; hop 3 runs in fp8e4m3 DoubleRow (0.5 cycles/row, 256-deep
    contraction per instruction) - its ~400-neighbor averaging keeps the fp8
    quantization error at ~8e-3 of global absmax. Hop 2 stays fp16 (its
    concentrated softmax would push fp8 error past the 2e-2 gate).
      * One 512-wide matmul per (j-chunk, batch) computes H for all four
        hops at once: stationary x^T chunk, moving [W0|W1|W2|W3]. The hop-0
        slice plus bias is directly out_0 (A_0 = I since m_0 = I).
      * out_k[i,(b,f)] += P_k^T-tile @ H-tile accumulated over j in PSUM.
      * Z row sums via ones-stationary 512-wide chains -> [1, n] PSUM, then
        transposed to per-partition layout with 8 one-hot outer-product
        matmuls (an SBUF AP cannot scatter its free dim across partitions,
        and a second DRAM output breaks the NEFF loader).
      * bias: hops 0,1 folded into H during h_build; hops 2,3 added in PSUM
        as a K=1 outer product Z (x) bias_k, since (P@H + Z*b)/Z = out + b.
        Evictions are then a pure 1/Z scale, splittable across DVE and ACT.
  - Engine split: PE matmuls / ACT exp + fp8 H copies + some evictions /
    DVE h_build bias-adds + most evictions + reciprocal.
  - One unified 2-bank PSUM pool tag (3 bufs) shared by h_build tiles and
    output accumulators + a small Z tag; exactly 8 banks.
  - Outputs staged fp16 (halves write traffic); host casts to fp32.
"""

import os
import sys

import numpy as np

sys.path.insert(0, "/opt/trn_rl_repo")
sys.path.insert(0, "/opt/trn_rl_repo/concourse")

import concourse.bass as bass  # noqa: E402
import concourse.mybir as mybir  # noqa: E402
import concourse.tile as tile  # noqa: E402
import concourse.bass_utils as _bu  # noqa: E402
import concourse.bass2jax as _b2j  # noqa: E402
from concourse.bass_utils import run_bass_kernel_spmd  # noqa: E402

# ---------------------------------------------------------------------------
# Workaround for this walrus build: the TRN2 ISA has exactly one sync-wait
# slot per 64B instruction, and this compiler errors ("Too many sync wait
# commands") instead of splitting multi-wait instructions emitted by Tile.
# Split them ourselves at the BIR-JSON level: hoist all but one wait onto
# single-wait NoOps inserted right before the instruction on the same engine
# queue (queue waits execute in order, so this is semantically identical).
# ---------------------------------------------------------------------------
import json as _json  # noqa: E402


def _split_multi_waits_json(bir_json):
    if isinstance(bir_json, (bytes, bytearray)):
        m = _json.loads(bir_json.decode())
    else:
        m = _json.loads(bir_json)
    ctr = 0
    for fn in m["functions"]:
        for blk in fn["blocks"]:
            out = []
            for inst in blk["instructions"]:
                si = inst.get("sync_info")
                if si:
                    ws = si.get("on_wait") or []
                    if len(ws) > 1:
                        for w in ws[:-1]:
                            ctr += 1
                            out.append(
                                {
                                    "debug": inst.get("debug", 0),
                                    "engine": inst["engine"],
                                    "ins": [],
                                    "name": f"WX-{ctr}",
                                    "opcode": "NoOp",
                                    "outs": [],
                                    "text_hint": "split_wait",
                                    "sync_info": {
                                        "on_update": [],
                                        "on_wait": [w],
                                    },
                                }
                            )
                        si["on_wait"] = [ws[-1]]
                    us = si.get("on_update") or []
                    if len(us) > 1:
                        raise RuntimeError(
                            f"multi-update inst {inst['name']}: unsupported"
                        )
                out.append(inst)
            blk["instructions"] = out
    return _json.dumps(m).encode()


_orig_compile_bir_kernel = _bu.compile_bir_kernel.__wrapped__ if hasattr(
    _bu.compile_bir_kernel, "__wrapped__"
) else _bu.compile_bir_kernel


def _patched_compile_bir_kernel(bir_json, tmpdir, neff_name="file.neff"):
    return _orig_compile_bir_kernel(
        _split_multi_waits_json(bir_json), tmpdir, neff_name
    )


_bu.compile_bir_kernel = _patched_compile_bir_kernel
if hasattr(_b2j, "compile_bir_kernel"):
    _b2j.compile_bir_kernel = _patched_compile_bir_kernel

N_CORES = 8
B = 64
N = 1024
F = 128
HOPS = 4
NEG = -9.0e15

# filled by kernel() for test.py to read
last_run_info = {}


def build_nc(b_local: int, n: int, f: int = 128, reps: int = 1, variant: str = ""):
    """Build the per-core Bass module.

    b_local: batches per core (8).  n: graph nodes.  f: feature dim (=128).
    """
    P = 128
    assert f == 128 and n % P == 0
    nch = n // P                # number of 128-row chunks (j and i)
    khops = HOPS - 1            # hops that need attention (1..3)
    bg = 4                      # batches per 512-wide eviction half
    n_half = b_local // bg      # halves of the (b,f) column space
    assert b_local == bg * n_half

    nc = bass.Bass()
    fp32 = mybir.dt.float32
    fp16 = mybir.dt.float16
    fp8 = mybir.dt.float8e4
    DR = mybir.MatmulPerfMode.DoubleRow
    xt_d = nc.dram_tensor("xt", [b_local, f, n], fp16, kind="ExternalInput")
    ep_d = nc.dram_tensor("ep", [khops, n, n], fp16, kind="ExternalInput")
    wc_d = nc.dram_tensor("wc", [f, HOPS * f], fp16, kind="ExternalInput")
    bias_d = nc.dram_tensor("bias", [HOPS * f], fp32, kind="ExternalInput")
    b16_d = nc.dram_tensor("bias16", [HOPS * f], fp16, kind="ExternalInput")
    out_d = nc.dram_tensor(
        "out", [nch, P, HOPS, b_local, f], fp16, kind="ExternalOutput"
    )

    with tile.TileContext(nc) as tc:
        with (
            tc.tile_pool(name="const", bufs=1) as const,
            tc.tile_pool(name="et", bufs=2) as etp,
            tc.tile_pool(name="et8", bufs=2) as et8p,
            tc.tile_pool(name="hh", bufs=1) as hhp,
            tc.tile_pool(name="stage", bufs=4) as stp,
            tc.tile_pool(name="zi", bufs=2) as zip_,
            tc.tile_pool(name="psA", bufs=3, space="PSUM") as psA,
        ):
            # ---- constants (loaded once, outside the reps loop) ----
            xt = const.tile([P, b_local, n], fp16)
            nc.sync.dma_start(out=xt, in_=xt_d[:].rearrange("b f j -> f b j"))
            wc = const.tile([P, HOPS * f], fp16)
            nc.sync.dma_start(out=wc, in_=wc_d[:])
            ones = const.tile([P, 8], fp16)
            nc.vector.memset(ones, 1.0)
            ones8 = const.tile([P, 2, 16], fp8)
            nc.vector.memset(ones8, 1.0)
            # one-hot rows for the PE-side Z transpose ([1,128] chunks -> zt cols)
            eye8 = const.tile([1, nch, nch], fp16)
            nc.vector.memset(eye8, 0.0)
            for c in range(nch):
                nc.vector.memset(eye8[:, c, c:c + 1], 1.0)
            # [bias0 | bias1] for the fused hop-0/1 h_build eviction (2 batches):
            # folding bias into H makes hop-1's eviction a pure 1/Z scale
            br01 = const.tile([P, 2, 2, f], fp32)
            for b2 in range(2):
                for kb in range(2):
                    bsl = bias_d[kb * f:(kb + 1) * f]
                    nc.sync.dma_start(
                        out=br01[:, b2, kb, :],
                        in_=bass.AP(
                            tensor=bsl.tensor, offset=bsl.offset,
                            ap=[[0, P], [1, f]],
                        ),
                    )
            # full bias replicated across partitions/batches (sttbias variant)
            br = const.tile([P, HOPS, b_local, f], fp32)
            for kb in range(HOPS):
                bsl = bias_d[kb * f:(kb + 1) * f]
                nc.sync.dma_start(
                    out=br[:, kb],
                    in_=bass.AP(
                        tensor=bsl.tensor, offset=bsl.offset,
                        ap=[[0, P], [0, b_local], [1, f]],
                    ),
                )
            # fp16 bias rows (hops 2,3) for the Z (x) bias outer-product
            bb16 = const.tile([1, 2, bg, f], fp16)
            for kb in range(2):
                bsl = b16_d[(kb + 2) * f:(kb + 3) * f]
                nc.sync.dma_start(
                    out=bb16[:, kb],
                    in_=bass.AP(
                        tensor=bsl.tensor, offset=bsl.offset,
                        ap=[[0, 1], [0, bg], [1, f]],
                    ),
                )

            for _rep in range(reps):
                def load_exp(kk):
                    """DMA E'_k^T (fp16, pre-masked) and exp it.

                    Hops 1,2 (kk=0,1): exp in place, fp16 (matmul dtype).
                    Hop 3 (kk=2): exp into an fp8e4 tile for DoubleRow.
                    """
                    et = etp.tile([P, nch, n], fp16, tag="et")
                    e8 = None if kk < 2 else et8p.tile(
                        [P, nch, n], fp8, tag="et8"
                    )
                    hc = nch // 2
                    for half in range(2):
                        sl = slice(half * hc, (half + 1) * hc)
                        nc.sync.dma_start(
                            out=et[:, sl],
                            in_=ep_d[kk, half * hc * P:(half + 1) * hc * P]
                            .rearrange("(c p) i -> p c i", p=P),
                        )
                        nc.scalar.activation(
                            out=(et if kk < 2 else e8)[:, sl],
                            in_=et[:, sl],
                            func=mybir.ActivationFunctionType.Exp,
                        )
                    return et if kk < 2 else e8

                if _rep == 0:
                    _pending = load_exp(0)
                ets = [None] * khops
                ets[0] = _pending

                def z_block(et, kk, on_act):
                    """Row-sums Z of P_k: ones-stationary chains -> [1, 512]
                    PSUM x2 -> fp16 copy (DVE or ACT) -> PE one-hot transpose
                    -> reciprocal. Returns (zs16, zinv8)."""
                    if "noz" in variant:
                        zs16 = zip_.tile([1, n], fp16, tag="zs")
                        nc.vector.memset(zs16, 1.0)
                        zinv8 = zip_.tile([P, nch], fp32, tag="zinv")
                        nc.vector.memset(zinv8, 1.0)
                        return zs16, zinv8
                    f8 = kk == 2
                    zps = [
                        psA.tile([1, n // 2], fp32, tag="z", bufs=2,
                                 name=f"zps_{kk}_{zz}")
                        for zz in range(2)
                    ]
                    if f8:
                        for jp in range(nch // 2):
                            st_, sp_ = (jp == 0), (jp == nch // 2 - 1)
                            for zz in range(2):
                                nc.tensor.matmul(
                                    zps[zz],
                                    ones8[:, :, 0:1],
                                    et[:, 2 * jp:2 * jp + 2,
                                       zz * (n // 2):(zz + 1) * (n // 2)],
                                    start=st_, stop=sp_, perf_mode=DR,
                                )
                    else:
                        for jc in range(nch):
                            st_, sp_ = (jc == 0), (jc == nch - 1)
                            for zz in range(2):
                                nc.tensor.matmul(
                                    zps[zz],
                                    ones[:, 0:1],
                                    et[:, jc, zz * (n // 2):(zz + 1) * (n // 2)],
                                    start=st_, stop=sp_,
                                )
                    zs16 = zip_.tile([1, n], fp16, tag="zs")
                    for zz in range(2):
                        sl = zs16[:, zz * (n // 2):(zz + 1) * (n // 2)]
                        if on_act:
                            nc.scalar.copy(out=sl, in_=zps[zz])
                        else:
                            nc.vector.tensor_scalar_add(sl, zps[zz], 0.0)
                    zinv8 = zip_.tile([P, nch], fp32, tag="zinv")
                    if "nozt" in variant:
                        nc.vector.memset(zinv8, 1.0)
                    else:
                        # transpose Z to per-partition layout on the PE: 8
                        # one-hot outer products accumulated into one PSUM
                        # region
                        zt = psA.tile([P, nch], fp32, tag="z", bufs=2,
                                      name=f"zt_{kk}")
                        for c in range(nch):
                            nc.tensor.matmul(
                                zt,
                                zs16[:, c * P:(c + 1) * P],
                                eye8[:, c],
                                start=(c == 0),
                                stop=(c == nch - 1),
                            )
                        nc.vector.reciprocal(out=zinv8, in_=zt)
                    return zs16, zinv8

                # hop-1 Z computed before h_build: PE only needs exp(E1),
                # and its fp16 staging lands early in the DVE queue
                zinfo = [None] * khops
                zinfo[0] = z_block(ets[0], 0, on_act=False)

                # ---- H for all hops in one 512-wide matmul per (jc, b);
                #      hop-0 slice (plus bias) is directly out_0, staged in
                #      hh; hop-1 H gets bias folded (pure-scale eviction);
                #      hop-2 H raw fp16; hop-3 H in fp8 for DoubleRow ----
                hh = hhp.tile([P, nch, b_local, 3, f], fp16, tag="hh")
                hh8 = hhp.tile([P, nch, b_local, f], fp8, tag="hh8")
                for jc in range(nch):
                    for bb in range(0, b_local, 2):
                        ps = psA.tile([P, 2 * HOPS * f], fp32, tag="A")
                        for db in range(2):
                            nc.tensor.matmul(
                                ps[:, db * HOPS * f:(db + 1) * HOPS * f],
                                xt[:, bb + db, jc * P:(jc + 1) * P],
                                wc,
                                start=True,
                                stop=True,
                            )
                        psv = ps.rearrange("p (b k f) -> p b k f", b=2, k=HOPS)
                        nc.vector.tensor_tensor(
                            out=hh[:, jc, bb:bb + 2, 0:2, :],
                            in0=psv[:, :, 0:2, :],
                            in1=br01,
                            op=mybir.AluOpType.add,
                        )
                        nc.scalar.copy(
                            out=hh[:, jc, bb:bb + 2, 2, :],
                            in_=psv[:, :, 2, :],
                        )
                        nc.scalar.copy(
                            out=hh8[:, jc, bb:bb + 2, :],
                            in_=psv[:, :, 3, :],
                        )
                    nc.sync.dma_start(
                        out=out_d[jc, :, 0], in_=hh[:, jc, :, 0, :]
                    )

                # ---- hops 1..3 ----
                for kk in range(khops):
                    if kk > 0:
                        # Z for this hop first (ACT-side staging for the fp8
                        # hop must precede the next exp in the ACT queue)
                        zinfo[kk] = z_block(ets[kk], kk, on_act=(kk == 2))
                    if kk + 1 < khops:
                        ets[kk + 1] = load_exp(kk + 1)
                    elif _rep + 1 < reps:
                        # prefetch next rep's hop-1 E during this rep's hop 3
                        _pending = load_exp(0)
                    et = ets[kk]
                    k = kk + 1
                    f8 = kk == 2
                    zs16, zinv8 = zinfo[kk]

                    has_outer = (kk > 0 and "noouter" not in variant
                                 and "sttbias" not in variant)
                    for ib in range(nch):
                        pos = psA.tile([P, b_local * f], fp32, tag="A",
                                       name=f"pos_{ib}")
                        if f8:
                            for jp in range(nch // 2):
                                lhsT = et[:, 2 * jp:2 * jp + 2,
                                          ib * P:(ib + 1) * P]
                                st_ = (jp == 0)
                                sp_ = (jp == nch // 2 - 1) and not has_outer
                                for h in range(n_half):
                                    nc.tensor.matmul(
                                        pos[:, h * bg * f:(h + 1) * bg * f],
                                        lhsT,
                                        hh8[:, 2 * jp:2 * jp + 2,
                                            h * bg:(h + 1) * bg, :],
                                        start=st_,
                                        stop=sp_,
                                        perf_mode=DR,
                                    )
                        else:
                            slot = 1 if kk == 0 else 2
                            for jc in range(nch):
                                lhsT = et[:, jc, ib * P:(ib + 1) * P]
                                st_ = (jc == 0)
                                sp_ = (jc == nch - 1) and not has_outer
                                for h in range(n_half):
                                    nc.tensor.matmul(
                                        pos[:, h * bg * f:(h + 1) * bg * f],
                                        lhsT,
                                        hh[:, jc, h * bg:(h + 1) * bg, slot, :],
                                        start=st_,
                                        stop=sp_,
                                    )
                        if has_outer:
                            # bias via Z (x) bias_k outer-product (K=1 matmul):
                            # (P@H + Z*b) / Z == out + b
                            for h in range(n_half):
                                nc.tensor.matmul(
                                    pos[:, h * bg * f:(h + 1) * bg * f],
                                    zs16[:, ib * P:(ib + 1) * P],
                                    bb16[:, kk - 1].rearrange("q a b -> q (a b)"),
                                    start=False,
                                    stop=True,
                                )
                        # eviction: pure 1/Z scale when bias is already in
                        # PSUM/H, else fused scale+bias stt (sttbias variant)
                        st = stp.tile([P, b_local * f], fp16, tag="stage")
                        if "sttbias" in variant and kk > 0:
                            nc.vector.scalar_tensor_tensor(
                                out=st,
                                in0=pos,
                                scalar=zinv8[:, ib:ib + 1],
                                in1=br[:, k].rearrange("p a b -> p (a b)"),
                                op0=mybir.AluOpType.mult,
                                op1=mybir.AluOpType.add,
                            )
                        elif ((kk == 2 and ib % 2 == 1)
                              or ("sttbias" in variant and ib % 2 == 1)):
                            nc.scalar.activation(
                                out=st,
                                in_=pos,
                                func=mybir.ActivationFunctionType.Copy,
                                scale=zinv8[:, ib:ib + 1],
                            )
                        else:
                            nc.vector.tensor_scalar_mul(
                                st, pos, zinv8[:, ib:ib + 1]
                            )
                        nc.sync.dma_start(
                            out=out_d[ib, :, k],
                            in_=st.rearrange("p (b f) -> p b f", b=b_local),
                        )
    return nc


_nc_cache = {}


def _get_nc(b_local, n, f):
    key = (b_local, n, f)
    if key not in _nc_cache:
        _nc_cache[key] = build_nc(b_local, n, f)
    return _nc_cache[key]


def _run(x, W, Es, bias, ms, n_cores, trace=False):
    """x:[B,N,F] W:[4,F,F] Es:[E1,E2,E3] ms:[m1,m2,m3] (hop-0 handled as identity)."""
    b, n, f = x.shape
    b_local = b // n_cores
    nc = _get_nc(b_local, n, f)

    # fold the mask into E on the host: exp(-100) == 0 exactly in fp16
    ep = np.ascontiguousarray(
        np.stack(
            [np.where(m, e, -100.0).T for e, m in zip(Es, ms)]
        ).astype(np.float16)
    )
    wc = np.ascontiguousarray(
        np.concatenate([W[k] for k in range(HOPS)], axis=1).astype(np.float16)
    )
    bias = np.ascontiguousarray(bias.astype(np.float32))

    in_maps = []
    for c in range(n_cores):
        xs = x[c * b_local:(c + 1) * b_local]          # [b_local, n, f]
        xts = np.ascontiguousarray(xs.transpose(0, 2, 1).astype(np.float16))
        in_maps.append({"xt": xts, "ep": ep, "wc": wc, "bias": bias,
                        "bias16": bias.astype(np.float16)})

    last_run_info["nc"] = nc
    last_run_info["in_maps"] = in_maps
    res = run_bass_kernel_spmd(
        nc, in_maps, core_ids=list(range(n_cores)), trace=trace
    )
    last_run_info["exec_time_ns"] = res.exec_time_ns
    last_run_info["trace"] = res.instructions_and_trace

    out = np.empty((b, n, HOPS * f), dtype=np.float32)
    for c in range(n_cores):
        od = res.results[c]["out"]          # [nch, P, HOPS, b_local, f]
        oc = od.transpose(3, 0, 1, 2, 4).reshape(b_local, n, HOPS * f)
        out[c * b_local:(c + 1) * b_local] = oc.astype(np.float32)
    return out


def build_null_nc(b_local: int, n: int, f: int = 128):
    """Same external tensors as build_nc but ~no device work — used to
    subtract host/transfer/dispatch overhead when estimating HW exec time."""
    P = 128
    khops = HOPS - 1
    nc = bass.Bass()
    fp32 = mybir.dt.float32
    fp16 = mybir.dt.float16
    nc.dram_tensor("xt", [b_local, f, n], fp16, kind="ExternalInput")
    nc.dram_tensor("ep", [khops, n, n], fp16, kind="ExternalInput")
    wc_d = nc.dram_tensor("wc", [f, HOPS * f], fp16, kind="ExternalInput")
    nc.dram_tensor("bias", [HOPS * f], fp32, kind="ExternalInput")
    nc.dram_tensor("bias16", [HOPS * f], fp16, kind="ExternalInput")
    nch = n // P
    out_d = nc.dram_tensor(
        "out", [nch, P, HOPS, b_local, f], fp16, kind="ExternalOutput"
    )
    with tile.TileContext(nc) as tc:
        with tc.tile_pool(name="p", bufs=1) as pool:
            t = pool.tile([P, 8], fp16)
            nc.sync.dma_start(out=t, in_=wc_d[:, 0:8])
            nc.sync.dma_start(out=out_d[0, :, 0, 0, 0:8], in_=t)
    return nc


def time_exec(iters=3):
    """Re-execute the last-run kernel and a null kernel; return
    (min_real_s, min_null_s). Uses identical input tensors so transfer and
    dispatch overhead cancels in the difference."""
    import time as _t

    nc = last_run_info["nc"]
    in_maps = last_run_info["in_maps"]
    n_cores = len(in_maps)
    reals, nulls = [], []
    for _ in range(iters):
        t0 = _t.time()
        run_bass_kernel_spmd(nc, in_maps, core_ids=list(range(n_cores)))
        reals.append(_t.time() - t0)
    b_local, f, n = in_maps[0]["xt"].shape
    nnc = build_null_nc(b_local, n, f)
    for _ in range(iters):
        t0 = _t.time()
        run_bass_kernel_spmd(nnc, in_maps, core_ids=list(range(n_cores)))
        nulls.append(_t.time() - t0)
    return min(reals), min(nulls), reals, nulls


def bench_exec(nc, in_maps, iters=10):
    """Device-resident repeated execution of the compiled kernel; returns
    per-call wall times (s) with inputs pre-staged on the 8 cores so only
    dispatch + device execution is measured."""
    import time as _t

    import jax
    import jax.numpy as jnp
    import mybir  # noqa: F401  # (ensure concourse paths set)
    from jax.experimental.shard_map import shard_map
    from jax.sharding import Mesh, PartitionSpec

    import concourse.mybir as mb
    from concourse import bass2jax as B

    B.install_neuronx_cc_hook()
    n_cores = len(in_maps)
    partition_name = (
        nc.partition_id_tensor.name if nc.partition_id_tensor else None
    )
    in_names, out_names, out_avals, zero_shapes = [], [], [], []
    for alloc in nc.m.functions[0].allocations:
        if not isinstance(alloc, mb.MemoryLocationSet):
            continue
        name = alloc.memorylocations[0].name
        if alloc.kind == "ExternalInput":
            if name != partition_name:
                in_names.append(name)
        elif alloc.kind == "ExternalOutput":
            shape = tuple(alloc.tensor_shape)
            dtype = mb.dt.np(alloc.dtype)
            out_names.append(name)
            out_avals.append(jax.core.ShapedArray(shape, dtype))
            zero_shapes.append((shape, dtype))
    n_params = len(in_names)
    all_in_names = list(in_names) + list(out_names)
    if partition_name is not None:
        all_in_names.append(partition_name)
    donate = tuple(range(n_params, n_params + len(out_names)))

    def _body(*args):
        operands = list(args)
        if partition_name is not None:
            operands.append(B.partition_id_tensor())
        outs = B._bass_exec_p.bind(
            *operands,
            out_avals=tuple(out_avals),
            in_names=tuple(all_in_names),
            out_names=tuple(out_names),
            lowering_input_output_aliases=(),
            sim_require_finite=True,
            sim_require_nnan=True,
            nc=nc,
        )
        return tuple(outs)

    devices = jax.devices()[:n_cores]
    mesh = Mesh(np.asarray(devices), ("core",))
    in_specs = (PartitionSpec("core"),) * (n_params + len(out_names))
    out_specs = (PartitionSpec("core"),) * len(out_names)
    fn = jax.jit(
        shard_map(
            _body, mesh=mesh, in_specs=in_specs, out_specs=out_specs,
            check_rep=False,
        ),
        donate_argnums=donate,
        keep_unused=True,
    )
    sh = jax.sharding.NamedSharding(mesh, PartitionSpec("core"))
    dev_in = [
        jax.device_put(
            np.concatenate([np.asarray(m[nm]) for m in in_maps], axis=0), sh
        )
        for nm in in_names
    ]

    def zeros():
        return [
            jax.device_put(
                jnp.zeros((n_cores * s[0],) + tuple(s[1:]), dt), sh
            )
            for (s, dt) in zero_shapes
        ]

    # warm up (compile + first exec)
    outs = fn(*dev_in, *zeros())
    jax.block_until_ready(outs)
    times = []
    for _ in range(iters):
        z = zeros()
        jax.block_until_ready(z)
        t0 = _t.perf_counter()
        outs = fn(*dev_in, *z)
        jax.block_until_ready(outs)
        times.append(_t.perf_counter() - t0)
    return times


def kernel(**inputs) -> np.ndarray:
    x = np.asarray(inputs["x"], dtype=np.float32)
    W = np.asarray(inputs["W"], dtype=np.float32)
    Es = [np.asarray(inputs[f"E{i}"], dtype=np.float32) for i in range(4)]
    bias = np.asarray(inputs["bias"], dtype=np.float32)
    ms = [np.asarray(inputs[f"m{i}"]).astype(bool) for i in range(4)]

    trace = bool(int(os.environ.get("HGRAPH_TRACE", "0")))
    out = _run(x, W, Es[1:], bias, ms[1:], N_CORES, trace=trace)

    f = W.shape[2]
    n = x.shape[1]
    # Safety net 1: hop 0 assumes m0 == I (structurally true for this module).
    if not np.array_equal(ms[0], np.eye(n, dtype=bool)):
        s0 = np.where(ms[0], Es[0], NEG)
        s0 = s0 - s0.max(axis=1, keepdims=True)
        p0 = np.exp(s0)
        a0 = p0 / p0.sum(axis=1, keepdims=True)
        h0 = np.einsum("bnf,fo->bno", x, W[0])
        out[:, :, 0:f] = np.einsum("ij,bjo->bio", a0, h0) + bias[None, None, :f]
    # Safety net 2: all-masked rows (softmax -> uniform; device would give NaN).
    for k in range(1, 4):
        empty = ~ms[k].any(axis=1)
        if empty.any():
            hk = np.einsum("bnf,fo->bno", x, W[k])
            unif = hk.mean(axis=1)  # [B, f]
            idx = np.where(empty)[0]
            out[:, idx, k * f:(k + 1) * f] = unif[:, None, :] + bias[None, None, k * f:(k + 1) * f]
    return out
